# revision 44
# baseline (speedup 1.0000x reference)
"""Multi-head attention (B=2, S=2048, D=1024, H=16) on 8 Trainium2 NeuronCores.

Sharding: batch x head-group. Core c handles batch b = c//4 and heads
[4*(c%4), 4*(c%4)+4) (a 256-wide slice of the QKV projection output and the
matching 256-row slice of Wo). Each core computes its partial output
projection; a 4-way ReduceScatter per batch group sums the partials and
writes each core's [128, 1024] row block of the final output directly, which
the host reassembles.

Per-core dataflow (all matmul operands fp16, fp32 PSUM accumulation):
  - x fed pre-blocked from the host as [8, 2048, 128] contiguous dt-blocks;
    DMA transposes issued as [1024, 128] halves (~2x the throughput of a
    full [2048, 128] transpose on the xbar path).
  - Q^T, K^T feature-major [256, 2048]; K^T zero-padded per head (full-128
    contraction); V token-major with per-head-pair blocks [v_h0|ones|v_h1]
    (192 cols) so each attn@V matmul (M=128, plain mode) also accumulates the
    softmax denominators on the 64 partitions opposite the attn rows - the
    per-k sums cost zero extra PE cycles and arrive pre-broadcast.
  - Softmax without max-subtraction (exp via ScalarE, 1/sqrt(dh) folded in);
    projection bias/copies are Vector tensor_scalar/tensor_tensor ops.
    Normalization crosses the 64-partition boundary with Scalar ACTIVATE
    copies - the only engine that can shift partition bases on HW (DVE ops
    cannot, and DMA would queue behind ReduceScatter SDMA traffic for
    10-50us at a time).
  - No tile_position anywhere: a single plain PE mode, no drain semaphores.
  - Software pipeline: each window front-loads ph2 attn@V of the previous
    window (norm(prev) then completes mid-window, freeing the acc banks
    before the next window needs them) interleaved with ph1 scores+exp of
    the current window; po(qc-1) runs at the start of pr1 windows and its
    per-chunk ReduceScatter fires mid-window.  The last q-chunk's attn@V
    runs as four 128-q sub-chunks in borrowed psB slots, each sub's norm and
    output projection trailing under the next sub's matmuls, so the final
    ReduceScatter fires as early as possible.
"""

import numpy as np

import concourse.bass as bass  # noqa: F401  (engine namespaces via nc)
import concourse.mybir as mybir
import concourse.tile as tile
from concourse import bacc
from concourse.bass import _add_dep_helper
from concourse.bass_utils import run_bass_kernel_spmd

F32 = mybir.dt.float32
F16 = mybir.dt.float16
AF = mybir.ActivationFunctionType

B, S, D = 2, 2048, 1024
H, DH = 16, 64
NCORES = 8
GPB = 4                # cores per batch group
HPC = H // GPB         # heads per core
DS = HPC * DH          # 256: per-core slice of the projection output
P = 128
NDT = D // P           # 8 d_model tiles
NTT = S // P           # 16 token tiles
QCH = 512              # q-chunk (PSUM bank = 512 fp32)
NQC = S // QCH         # 4
NKT = S // P           # 16 k tiles
VW = 192               # per-head-pair V block: [v_h0 | ones | v_h1]
SCALE = float(1.0 / np.sqrt(DH))

REPLICA_GROUPS = [[0, 1, 2, 3], [4, 5, 6, 7]]

_CACHED_NC = None


def _build_module(with_cc=True):
    nc = bacc.Bacc("TRN2", target_bir_lowering=False, debug=False,
                   num_devices=NCORES)

    xq_d = nc.dram_tensor("xq", [NDT, S, P], F16, kind="ExternalInput")
    xk_d = nc.dram_tensor("xk", [NDT, S, P], F16, kind="ExternalInput")
    xv_d = nc.dram_tensor("xv", [NDT, S, P], F16, kind="ExternalInput")
    wq_d = nc.dram_tensor("wq", [D, DS], F16, kind="ExternalInput")
    wk_d = nc.dram_tensor("wk", [D, DS], F16, kind="ExternalInput")
    wv_d = nc.dram_tensor("wv", [D, DS], F16, kind="ExternalInput")
    wo_d = nc.dram_tensor("wo", [DS, D], F16, kind="ExternalInput")
    bq_d = nc.dram_tensor("bq", [DS, 1], F32, kind="ExternalInput")
    bk_d = nc.dram_tensor("bk", [DS, 1], F32, kind="ExternalInput")
    bv_d = nc.dram_tensor("bv", [1, DS], F32, kind="ExternalInput")
    bo_d = nc.dram_tensor("bo", [1, D], F32, kind="ExternalInput")

    out_d = nc.dram_tensor("out", [S // GPB, D], F16, kind="ExternalOutput")
    partial_cs = [nc.dram_tensor(f"partial{j}", [4 * P, D], F16)
                  for j in range(4)]
    rs_cs = [nc.dram_tensor(f"rs_out{j}", [P, D], F16)
             for j in range(3)]
    rs3h_cs = [nc.dram_tensor(f"rs3h{j}", [P // 2, D], F16)
               for j in range(2)]


    with tile.TileContext(nc) as tc:
        with (
            tc.tile_pool(name="cst", bufs=1) as cst,
            tc.tile_pool(name="xt", bufs=12) as xtp,
            tc.tile_pool(name="exp", bufs=26) as expp,
            tc.tile_pool(name="rcp", bufs=2) as rcpp,
            tc.tile_pool(name="osb", bufs=8) as osbp,
            tc.tile_pool(name="psB", bufs=3, space="PSUM") as psB,
            tc.tile_pool(name="psC", bufs=1, space="PSUM") as psC,
        ):
            # Total PE ordering: chain every matmul to its predecessor
            # (nosync = scheduling-order only). All matmuls are plain
            # 128x128 mode - no tile_position, no mode-switch drains.
            _real_matmul = nc.tensor.matmul
            _prev_mm = {"inst": None}

            def mm(out, lhsT, rhs, **kw):
                inst = _real_matmul(out, lhsT, rhs, **kw)
                if _prev_mm["inst"] is not None:
                    _add_dep_helper(
                        inst.ins, _prev_mm["inst"].ins,
                        sync=False, reason="pe-order")
                _prev_mm["inst"] = inst
                return inst

            # ---- constants (sync HWDGE queue, wk first) ----
            wq_t = cst.tile([P, NDT, DS], F16, tag="wq")
            wk_t = cst.tile([P, NDT, DS], F16, tag="wk")
            wv_t = cst.tile([P, NDT, DS], F16, tag="wv")
            wo_t = cst.tile([P, 2, D], F16, tag="wo")
            bq_t = cst.tile([P, 2, 1], F32, tag="bq")
            bk_t = cst.tile([P, 2, 1], F32, tag="bk")
            bv_row = cst.tile([1, DS], F32, tag="bvr")
            bo_row = cst.tile([1, D], F32, tag="bor")

            # wo/bo are not needed until the first output projection
            # (~115us in) - load them AFTER the transposes so the transpose
            # stream starts ~1.5us earlier.
            nc.sync.dma_start(wk_t[:], wk_d.rearrange("(a p) n -> p a n", p=P))
            nc.sync.dma_start(wq_t[:], wq_d.rearrange("(a p) n -> p a n", p=P))
            nc.sync.dma_start(bk_t[:], bk_d.rearrange("(a p) o -> p a o", p=P))
            nc.sync.dma_start(bq_t[:], bq_d.rearrange("(a p) o -> p a o", p=P))
            nc.sync.dma_start(bv_row[:], bv_d[:])
            nc.sync.dma_start(wv_t[:], wv_d.rearrange("(a p) n -> p a n", p=P))

            bv_b = cst.tile([P, DS], F32, tag="bvb")
            bo_b = cst.tile([P, D], F32, tag="bob")
            nc.gpsimd.partition_broadcast(bv_b[:], bv_row[:])

            # ---- activations: resident projections ----
            qt_t = cst.tile([P, 2, S], F16, tag="qt")   # Q^T  (pair, t)
            # K^T zero-padded per head: kz[:, h, :] has rows (h%2)*64..+64 =
            # K_h^T, other 64 rows zero -> full-K=128 scores matmuls.
            kz_t = cst.tile([P, HPC, S], F16, tag="kz")
            nc.vector.memset(kz_t[:], 0.0)
            # V token-major, per pair [v_h0 | ones | v_h1] (ones shared)
            v2_t = cst.tile([P, NTT, 2, VW], F16, tag="vt")
            nc.vector.memset(v2_t[:, :, :, DH:2 * DH], 1.0)
            an_t = cst.tile([P, 2, S], F16, tag="an")   # attn/sum ratio ^T

            # ---- transposed input tiles (DMA transpose, fp16) ----
            # [1024, 128] half-transposes run ~2x the throughput of full
            # [2048, 128] ones (1.30us vs ~2.9us per half-pair).
            def load_xt(x_d, half_major=False):
                tiles = [xtp.tile([P, S], F16, tag="xt", name=f"xt{dt}")
                         for dt in range(NDT)]
                hs = ([(h, dt) for h in range(2) for dt in range(NDT)]
                      if half_major else
                      [(h, dt) for dt in range(NDT) for h in range(2)])
                for h, dt in hs:
                    nc.sync.dma_start(
                        tiles[dt][:, h * 1024:(h + 1) * 1024],
                        x_d[dt, h * 1024:(h + 1) * 1024, :], transpose=True)
                return tiles

            xt_k = load_xt(xk_d)
            xt_q = load_xt(xq_d)
            xt_v = load_xt(xv_d, half_major=True)

            nc.sync.dma_start(wo_t[:], wo_d.rearrange("(a p) n -> p a n", p=P))
            nc.sync.dma_start(bo_row[:], bo_d[:])
            nc.gpsimd.partition_broadcast(bo_b[:], bo_row[:])

            # ---- feature-major projection: out^T[ds, t] (Q^T / K^T) ----
            # dt-interleaved across up to 3 q-chunks (3 PSUM tiles live) so
            # each transposed tile is consumed the moment it lands.
            def proj_T_group(dst, w_t, b_t, xt, tcis):
                pss = {}
                for tci in tcis:
                    ps = psB.tile([P, 2 * QCH], F32, tag="sc", name=f"ps{tci}")
                    pss[tci] = ps
                for dt in range(NDT):
                    for tci in tcis:
                        ts0 = tci * QCH
                        for dot in range(2):
                            col = slice(dot * QCH, (dot + 1) * QCH)
                            mm(
                                pss[tci][:, col],
                                w_t[:, dt, dot * P:(dot + 1) * P],
                                xt[dt][:, ts0:ts0 + QCH],
                                start=(dt == 0), stop=(dt == NDT - 1),
                            )
                for tci in tcis:
                    ps = pss[tci]
                    ts0 = tci * QCH
                    if dst is qt_t:
                        for dot in range(2):
                            nc.vector.tensor_scalar_add(
                                dst[:, dot, ts0:ts0 + QCH],
                                ps[:, dot * QCH:(dot + 1) * QCH],
                                b_t[:, dot, :])
                    else:  # kz_t: per-head 64-row slices, rest stays zero
                        for h in range(HPC):
                            rows = slice((h % 2) * 64, (h % 2) * 64 + 64)
                            dot = h // 2
                            nc.vector.tensor_scalar_add(
                                kz_t[rows, h, ts0:ts0 + QCH],
                                ps[rows, dot * QCH:(dot + 1) * QCH],
                                b_t[rows, dot, :])

            # ---- token-major V projection (one tile) ----
            # write head 2pr to pair-block cols 0:64, head 2pr+1 to 128:192
            def proj_v(tt):
                ps = psB.tile([P, DS], F32, tag="sc")
                for dt in range(NDT):
                    mm(
                        ps[:],
                        xt_v[dt][:, tt * P:(tt + 1) * P],
                        wv_t[:, dt, :],
                        start=(dt == 0), stop=(dt == NDT - 1),
                    )
                psv = ps.rearrange("p (pr h d) -> p pr h d", pr=2, h=2)
                bvv = bv_b.rearrange("p (pr h d) -> p pr h d", pr=2, h=2)
                nc.vector.tensor_add(
                    v2_t[:, tt, :, 0:DH], psv[:, :, 0, :], bvv[:, :, 0, :])
                nc.vector.tensor_add(
                    v2_t[:, tt, :, 2 * DH:VW], psv[:, :, 1, :], bvv[:, :, 1, :])

            # ---- attention phases ----
            # scores + exp for one kp step (2 k-blocks x 2 heads, plain)
            def ph1_kp(qc, pr, kp):
                qs = qc * QCH
                h0, h1 = 2 * pr, 2 * pr + 1
                sc0 = psB.tile([P, 2 * QCH], F32, tag="sc", name="sc0")
                sc1 = psB.tile([P, 2 * QCH], F32, tag="sc", name="sc1")
                for hsel, sc in ((h0, sc0), (h1, sc1)):
                    for j in range(2):
                        ks = (2 * kp + j) * P
                        col = slice(j * QCH, (j + 1) * QCH)
                        mm(
                            sc[:, col], kz_t[:, hsel, ks:ks + P],
                            qt_t[:, pr, qs:qs + QCH],
                            start=True, stop=True)
                e0 = expp.tile([P, 2 * QCH], F16, tag="exp", name="e0")
                e1 = expp.tile([P, 2 * QCH], F16, tag="exp", name="e1")
                nc.scalar.activation(e0[:], sc0[:], AF.Exp, scale=SCALE)
                nc.scalar.activation(e1[:], sc1[:], AF.Exp, scale=SCALE)
                return (e0, e1)

            # attn@V + free sums for a subset of k tiles, accumulating into
            # acc0/acc1 across the whole 16-kt sweep.  Plain M=128 matmuls:
            #   acc0 rows 0:64 = attn h0, rows 64:128 = sum(e0) per q
            #   acc1 rows 0:64 = sum(e1) per q, rows 64:128 = attn h1
            def ph2_chunk(qc, pr, etiles, kts, acc0, acc1):
                for kt in kts:
                    e0, e1 = etiles[kt // 2]
                    col = slice((kt % 2) * QCH, (kt % 2 + 1) * QCH)
                    st = (kt == 0)
                    sp = (kt == NKT - 1)
                    mm(acc0[:], v2_t[:, kt, pr, 0:P], e0[:, col],
                       start=st, stop=sp)
                    mm(acc1[:], v2_t[:, kt, pr, DH:DH + P], e1[:, col],
                       start=st, stop=sp)

            # DVE ops cannot cross partition bases on HW (and the custom
            # reciprocal only works at base 0), but the Scalar engine's
            # ACTIVATE-copy crosses bases fine and reads PSUM directly.
            # Crucially this keeps DMA out of the norm path - collective
            # (ReduceScatter) traffic hogs the DMA hardware for ~12us at a
            # time and would stall the pipeline behind it.
            def _norm_w(pr, qs, w, acc0, acc1):
                rc = rcpp.tile([P, QCH], F32, tag="rcp", name="rc")
                rc2 = rcpp.tile([P, QCH], F32, tag="rcp2", name="rc2")
                rc3 = rcpp.tile([P, QCH], F32, tag="rcp3", name="rc3")
                # h1 first: recip at base 0 straight from psum, then the
                # Scalar copy moves it up while h0's chain proceeds.
                nc.vector.reciprocal_approx_fast(rc3[0:64, 0:w],
                                                 acc1[0:64, :])
                nc.scalar.copy(rc3[64:P, 0:w], rc3[0:64, 0:w])
                # h0: ACT-copy sums down (psum rows 64:128 -> sbuf rows 0:64)
                nc.scalar.copy(rc2[0:64, 0:w], acc0[64:P, :])
                nc.vector.reciprocal_approx_fast(rc[0:64, 0:w],
                                                 rc2[0:64, 0:w])
                nc.vector.tensor_mul(
                    an_t[0:64, pr, qs:qs + w], acc0[0:64, :], rc[0:64, 0:w])
                nc.vector.tensor_mul(
                    an_t[64:P, pr, qs:qs + w], acc1[64:P, :], rc3[64:P, 0:w])

            def norm(qc, pr, acc0, acc1):
                _norm_w(pr, qc * QCH, QCH, acc0, acc1)

            def norm_sub(qc, pr, sub, a0, a1):
                """norm for one 128-q sub-chunk of the tail."""
                _norm_w(pr, qc * QCH + sub * P, P, a0, a1)

            def po_tt(qc, tt4):
                """one token tile of the output projection (plain matmuls)."""
                tt = qc * (QCH // P) + tt4
                po = psB.tile([P, 2 * QCH], F32, tag="sc")
                for half in range(2):
                    for pr in range(2):
                        mm(
                            po[:, half * QCH:(half + 1) * QCH],
                            an_t[:, pr, tt * P:(tt + 1) * P],
                            wo_t[:, pr, half * QCH:(half + 1) * QCH],
                            start=(pr == 0), stop=(pr == 1))
                ob = osbp.tile([P, D], F16, tag="osb")
                nc.vector.tensor_add(ob[:], po[:], bo_b[:])
                nc.sync.dma_start(
                    partial_cs[tt // 4][(tt % 4) * P:(tt % 4 + 1) * P, :],
                    ob[:])

            def rs_qc(qc):
                """per-chunk ReduceScatter (out DMAs issued at the end)."""
                if not with_cc:
                    return
                nc.gpsimd.collective_compute(
                    "ReduceScatter",
                    mybir.AluOpType.add,
                    replica_groups=REPLICA_GROUPS,
                    ins=[partial_cs[qc][:]],
                    outs=[rs_cs[qc][:]],
                )

            def out_dma(qc):
                if not with_cc:
                    return
                nc.sync.dma_start(out_d[qc * P:(qc + 1) * P, :],
                                  rs_cs[qc][:])

            def rs3_half(h):
                """qc=3 ReduceScatter in [256, D] halves: the first fires
                while the tail's remaining sub-chunks still compute."""
                if not with_cc:
                    return
                nc.gpsimd.collective_compute(
                    "ReduceScatter",
                    mybir.AluOpType.add,
                    replica_groups=REPLICA_GROUPS,
                    ins=[partial_cs[3][h * 2 * P:(h + 1) * 2 * P, :]],
                    outs=[rs3h_cs[h][:]],
                )
                nc.sync.dma_start(
                    out_d[3 * P + h * 64:3 * P + (h + 1) * 64, :],
                    rs3h_cs[h][:])



            # ---- emission schedule ----
            proj_T_group(kz_t, wk_t, bk_t, xt_k, [0, 1, 2])
            proj_T_group(kz_t, wk_t, bk_t, xt_k, [3])
            proj_T_group(qt_t, wq_t, bq_t, xt_q, [0, 1, 2])
            proj_T_group(qt_t, wq_t, bq_t, xt_q, [3])

            # first q-chunk's pr0 scores run while V transposes stream in
            et00 = [ph1_kp(0, 0, kp) for kp in range(NKT // 2)]
            for tt in range(NTT):
                proj_v(tt)



            # software pipeline: each window FRONT-LOADS ph2(prev) so that
            # norm(prev) completes mid-window - the next window's first
            # attn@V matmul (which reuses the acc banks) then never stalls.
            # ph1(cur) kp0-3 interleave to keep the Scalar exp stream fed;
            # kp4-7 + the output projection of qc-1 fill the window's tail.
            prev = (0, 0, et00)
            seq = [(qc, pr) for qc in range(NQC) for pr in range(2)][1:]
            for qc, pr in seq:
                acc0 = psC.tile([P, QCH], F32, tag="acc0", name="acc0")
                acc1 = psC.tile([P, QCH], F32, tag="acc1", name="acc1")
                # po(qc-1) runs at the START of the pr1 window (its an_t was
                # normalized a full window earlier - no wait) and its
                # ReduceScatter fires mid-window, keeping collectives off the
                # kernel tail.
                po_src = qc - 1 if (pr == 1 and qc >= 1) else None
                et = []
                et.append(ph1_kp(qc, pr, 0))
                if po_src is not None:
                    po_tt(po_src, 0)
                    po_tt(po_src, 1)
                et.append(ph1_kp(qc, pr, 1))
                if po_src is not None:
                    po_tt(po_src, 2)
                    po_tt(po_src, 3)
                ph2_chunk(prev[0], prev[1], prev[2], range(0, 8), acc0, acc1)
                et.append(ph1_kp(qc, pr, 2))
                if po_src is not None:
                    rs_qc(po_src)
                ph2_chunk(prev[0], prev[1], prev[2], range(8, 16), acc0, acc1)
                et.append(ph1_kp(qc, pr, 3))
                norm(prev[0], prev[1], acc0, acc1)
                for kp in range(4, 8):
                    et.append(ph1_kp(qc, pr, kp))
                prev = (qc, pr, et)

            # tail: the last chunk's attn@V in four 128-q sub-chunks, each
            # immediately normalized + output-projected + reduce-scattered
            # (per-token-tile RS quarters the final collective's latency).
            # Sub-accumulators borrow psB slots: [:, 0:128] of bank A and
            # [QCH:QCH+128] of bank B - start=True zeroing is bank-granular
            # and both banks belong solely to the slot.
            SW = 256  # tail sub-chunk width
            subacc = []
            for sub in range(2):
                slot = psB.tile([P, 2 * QCH], F32, tag="sc", name=f"fs{sub}")
                a0 = slot[:, 0:SW]
                a1 = slot[:, QCH:QCH + SW]
                for kt in range(NKT):
                    e0, e1 = prev[2][kt // 2]
                    col = slice((kt % 2) * QCH + sub * SW,
                                (kt % 2) * QCH + (sub + 1) * SW)
                    st = (kt == 0)
                    sp = (kt == NKT - 1)
                    mm(a0, v2_t[:, kt, 1, 0:P], e0[:, col],
                       start=st, stop=sp)
                    mm(a1, v2_t[:, kt, 1, DH:DH + P], e1[:, col],
                       start=st, stop=sp)
                subacc.append((a0, a1))
            # norms/po/RS trail under the second sub's matmuls
            _norm_w(1, 3 * QCH, SW, *subacc[0])
            po_tt(3, 0)
            po_tt(3, 1)
            rs3_half(0)
            _norm_w(1, 3 * QCH + SW, SW, *subacc[1])
            po_tt(3, 2)
            po_tt(3, 3)
            rs3_half(1)
            out_dma(0)
            out_dma(1)
            out_dma(2)

    nc.compile()
    return nc


def _get_nc():
    global _CACHED_NC
    if _CACHED_NC is None:
        _CACHED_NC = _build_module()
    return _CACHED_NC


def _block_x(x):
    """[S, D] fp32 -> [NDT, S, P] fp16 contiguous dt-blocks."""
    return np.ascontiguousarray(
        x.reshape(S, NDT, P).transpose(1, 0, 2)).astype(np.float16)


def _make_in_maps(query, key, value, Wq, bq, Wk, bk, Wv, bv, Wo, bo):
    query = np.asarray(query, dtype=np.float32)
    key = np.asarray(key, dtype=np.float32)
    value = np.asarray(value, dtype=np.float32)
    Wq = np.asarray(Wq, dtype=np.float32)
    Wk = np.asarray(Wk, dtype=np.float32)
    Wv = np.asarray(Wv, dtype=np.float32)
    Wo = np.asarray(Wo, dtype=np.float32)
    bq = np.asarray(bq, dtype=np.float32)
    bk = np.asarray(bk, dtype=np.float32)
    bv = np.asarray(bv, dtype=np.float32)
    bo = np.asarray(bo, dtype=np.float32)

    xb = [(_block_x(query[b]), _block_x(key[b]), _block_x(value[b]))
          for b in range(B)]

    in_maps = []
    for c in range(NCORES):
        b = c // GPB
        g = c % GPB
        sl = slice(g * DS, (g + 1) * DS)
        in_maps.append({
            "xq": xb[b][0],
            "xk": xb[b][1],
            "xv": xb[b][2],
            "wq": Wq[:, sl].astype(np.float16),
            "wk": Wk[:, sl].astype(np.float16),
            "wv": Wv[:, sl].astype(np.float16),
            "wo": Wo[sl, :].astype(np.float16),
            "bq": bq[sl].reshape(DS, 1).copy(),
            "bk": bk[sl].reshape(DS, 1).copy(),
            "bv": bv[sl].reshape(1, DS).copy(),
            "bo": (bo if g == 0 else np.zeros_like(bo)).reshape(1, D).copy(),
        })
    return in_maps


def run(inputs, trace=False, trace_cores=None):
    """Run the SPMD kernel; returns (full_output, BassKernelResults)."""
    nc = _get_nc()
    in_maps = _make_in_maps(**inputs)
    res = run_bass_kernel_spmd(
        nc, in_maps, core_ids=list(range(NCORES)), trace=trace,
        trace_cores=trace_cores)
    out = np.empty((B, S, D), dtype=np.float32)
    for c in range(NCORES):
        b = c // GPB
        g = c % GPB
        o = res.results[c]["out"].astype(np.float32)
        for j in range(3):
            out[b, j * 512 + g * P:j * 512 + (g + 1) * P, :] = \
                o[j * P:(j + 1) * P, :]
        # qc=3 was reduce-scattered in two [256, D] halves (64 rows/core)
        for h in range(2):
            out[b, 3 * 512 + h * 256 + g * 64:
                   3 * 512 + h * 256 + (g + 1) * 64, :] = \
                o[3 * P + h * 64:3 * P + (h + 1) * 64, :]
    return out, res


def kernel(**inputs):
    out, _ = run(inputs, trace=False)
    return out


# revision 45
# speedup vs baseline: 1.0224x; 1.0224x over previous
"""Multi-head attention (B=2, S=2048, D=1024, H=16) on 8 Trainium2 NeuronCores.

Sharding: batch x head-group. Core c handles batch b = c//4 and heads
[4*(c%4), 4*(c%4)+4) (a 256-wide slice of the QKV projection output and the
matching 256-row slice of Wo). Each core computes its partial output
projection; a 4-way ReduceScatter per batch group sums the partials and
writes each core's [128, 1024] row block of the final output directly, which
the host reassembles.

Per-core dataflow (all matmul operands fp16, fp32 PSUM accumulation):
  - x fed pre-blocked from the host as [8, 2048, 128] contiguous dt-blocks;
    DMA transposes issued as [1024, 128] halves (~2x the throughput of a
    full [2048, 128] transpose on the xbar path).
  - Q^T, K^T feature-major [256, 2048]; K^T zero-padded per head (full-128
    contraction); V token-major with per-head-pair blocks [v_h0|ones|v_h1]
    (192 cols) so each attn@V matmul (M=128, plain mode) also accumulates the
    softmax denominators on the 64 partitions opposite the attn rows - the
    per-k sums cost zero extra PE cycles and arrive pre-broadcast.
  - Softmax without max-subtraction (exp via ScalarE, 1/sqrt(dh) folded in);
    projection bias/copies are Vector tensor_scalar/tensor_tensor ops.
    Normalization crosses the 64-partition boundary with Scalar ACTIVATE
    copies - the only engine that can shift partition bases on HW (DVE ops
    cannot, and DMA would queue behind ReduceScatter SDMA traffic for
    10-50us at a time).
  - No tile_position anywhere: a single plain PE mode, no drain semaphores.
  - Software pipeline: each window front-loads ph2 attn@V of the previous
    window (norm(prev) then completes mid-window, freeing the acc banks
    before the next window needs them) interleaved with ph1 scores+exp of
    the current window; po(qc-1) runs at the start of pr1 windows and its
    per-chunk ReduceScatter fires mid-window.  The last q-chunk's attn@V
    runs as four 128-q sub-chunks in borrowed psB slots, each sub's norm and
    output projection trailing under the next sub's matmuls, so the final
    ReduceScatter fires as early as possible.
"""

import numpy as np

import concourse.bass as bass  # noqa: F401  (engine namespaces via nc)
import concourse.mybir as mybir
import concourse.tile as tile
from concourse import bacc
from concourse.bass import _add_dep_helper
from concourse.bass_utils import run_bass_kernel_spmd

F32 = mybir.dt.float32
F16 = mybir.dt.float16
AF = mybir.ActivationFunctionType

B, S, D = 2, 2048, 1024
H, DH = 16, 64
NCORES = 8
GPB = 4                # cores per batch group
HPC = H // GPB         # heads per core
DS = HPC * DH          # 256: per-core slice of the projection output
P = 128
NDT = D // P           # 8 d_model tiles
NTT = S // P           # 16 token tiles
QCH = 512              # q-chunk (PSUM bank = 512 fp32)
NQC = S // QCH         # 4
NKT = S // P           # 16 k tiles
VW = 192               # per-head-pair V block: [v_h0 | ones | v_h1]
SCALE = float(1.0 / np.sqrt(DH))

REPLICA_GROUPS = [[0, 1, 2, 3], [4, 5, 6, 7]]

_CACHED_NC = None


def _build_module(with_cc=True):
    nc = bacc.Bacc("TRN2", target_bir_lowering=False, debug=False,
                   num_devices=NCORES)

    xq_d = nc.dram_tensor("xq", [NDT, S, P], F16, kind="ExternalInput")
    xk_d = nc.dram_tensor("xk", [NDT, S, P], F16, kind="ExternalInput")
    xv_d = nc.dram_tensor("xv", [NDT, S, P], F16, kind="ExternalInput")
    wq_d = nc.dram_tensor("wq", [D, DS], F16, kind="ExternalInput")
    wk_d = nc.dram_tensor("wk", [D, DS], F16, kind="ExternalInput")
    wv_d = nc.dram_tensor("wv", [D, DS], F16, kind="ExternalInput")
    wo_d = nc.dram_tensor("wo", [DS, D], F16, kind="ExternalInput")
    bq_d = nc.dram_tensor("bq", [DS, 1], F32, kind="ExternalInput")
    bk_d = nc.dram_tensor("bk", [DS, 1], F32, kind="ExternalInput")
    bv_d = nc.dram_tensor("bv", [1, DS], F32, kind="ExternalInput")
    bo_d = nc.dram_tensor("bo", [1, D], F32, kind="ExternalInput")

    out_d = nc.dram_tensor("out", [S // GPB, D], F16, kind="ExternalOutput")
    partial_cs = [nc.dram_tensor(f"partial{j}", [4 * P, D], F16)
                  for j in range(4)]
    rs_cs = [nc.dram_tensor(f"rs_out{j}", [P, D], F16)
             for j in range(3)]
    rs3h_cs = [nc.dram_tensor(f"rs3h{j}", [P // 2, D], F16)
               for j in range(2)]


    with tile.TileContext(nc) as tc:
        with (
            tc.tile_pool(name="cst", bufs=1) as cst,
            tc.tile_pool(name="xt", bufs=12) as xtp,
            tc.tile_pool(name="exp", bufs=26) as expp,
            tc.tile_pool(name="rcp", bufs=2) as rcpp,
            tc.tile_pool(name="osb", bufs=8) as osbp,
            tc.tile_pool(name="psB", bufs=3, space="PSUM") as psB,
            tc.tile_pool(name="psC", bufs=1, space="PSUM") as psC,
        ):
            # Total PE ordering: chain every matmul to its predecessor
            # (nosync = scheduling-order only). All matmuls are plain
            # 128x128 mode - no tile_position, no mode-switch drains.
            _real_matmul = nc.tensor.matmul
            _prev_mm = {"inst": None}

            def mm(out, lhsT, rhs, **kw):
                inst = _real_matmul(out, lhsT, rhs, **kw)
                if _prev_mm["inst"] is not None:
                    _add_dep_helper(
                        inst.ins, _prev_mm["inst"].ins,
                        sync=False, reason="pe-order")
                _prev_mm["inst"] = inst
                return inst

            # ---- constants (sync HWDGE queue, wk first) ----
            wq_t = cst.tile([P, NDT, DS], F16, tag="wq")
            wk_t = cst.tile([P, NDT, DS], F16, tag="wk")
            wv_t = cst.tile([P, NDT, DS], F16, tag="wv")
            wo_t = cst.tile([P, 2, D], F16, tag="wo")
            bq_t = cst.tile([P, 2, 1], F32, tag="bq")
            bk_t = cst.tile([P, 2, 1], F32, tag="bk")
            bv_row = cst.tile([1, DS], F32, tag="bvr")
            bo_row = cst.tile([1, D], F32, tag="bor")

            # wo/bo are not needed until the first output projection
            # (~115us in) - load them AFTER the transposes so the transpose
            # stream starts ~1.5us earlier.
            nc.sync.dma_start(wk_t[:], wk_d.rearrange("(a p) n -> p a n", p=P))
            nc.sync.dma_start(wq_t[:], wq_d.rearrange("(a p) n -> p a n", p=P))
            nc.sync.dma_start(bk_t[:], bk_d.rearrange("(a p) o -> p a o", p=P))
            nc.sync.dma_start(bq_t[:], bq_d.rearrange("(a p) o -> p a o", p=P))
            nc.sync.dma_start(bv_row[:], bv_d[:])
            nc.sync.dma_start(wv_t[:], wv_d.rearrange("(a p) n -> p a n", p=P))

            bv_b = cst.tile([P, DS], F32, tag="bvb")
            bo_b = cst.tile([P, D], F32, tag="bob")
            nc.gpsimd.partition_broadcast(bv_b[:], bv_row[:])

            # ---- activations: resident projections ----
            qt_t = cst.tile([P, 2, S], F16, tag="qt")   # Q^T  (pair, t)
            # K^T zero-padded per head: kz[:, h, :] has rows (h%2)*64..+64 =
            # K_h^T, other 64 rows zero -> full-K=128 scores matmuls.
            kz_t = cst.tile([P, HPC, S], F16, tag="kz")
            nc.vector.memset(kz_t[:], 0.0)
            # V token-major, per pair [v_h0 | ones | v_h1] (ones shared)
            v2_t = cst.tile([P, NTT, 2, VW], F16, tag="vt")
            nc.vector.memset(v2_t[:, :, :, DH:2 * DH], 1.0)
            an_t = cst.tile([P, 2, S], F16, tag="an")   # attn/sum ratio ^T

            # ---- transposed input tiles (DMA transpose, fp16) ----
            # [1024, 128] half-transposes run ~2x the throughput of full
            # [2048, 128] ones (1.30us vs ~2.9us per half-pair).
            def load_xt(x_d, half_major=False):
                tiles = [xtp.tile([P, S], F16, tag="xt", name=f"xt{dt}")
                         for dt in range(NDT)]
                hs = ([(h, dt) for h in range(2) for dt in range(NDT)]
                      if half_major else
                      [(h, dt) for dt in range(NDT) for h in range(2)])
                for h, dt in hs:
                    nc.sync.dma_start(
                        tiles[dt][:, h * 1024:(h + 1) * 1024],
                        x_d[dt, h * 1024:(h + 1) * 1024, :], transpose=True)
                return tiles

            xt_k = load_xt(xk_d)
            xt_q = load_xt(xq_d)
            xt_v = load_xt(xv_d, half_major=True)

            nc.sync.dma_start(wo_t[:], wo_d.rearrange("(a p) n -> p a n", p=P))
            nc.sync.dma_start(bo_row[:], bo_d[:])
            nc.gpsimd.partition_broadcast(bo_b[:], bo_row[:])

            # ---- feature-major projection: out^T[ds, t] (Q^T / K^T) ----
            # dt-interleaved across up to 3 q-chunks (3 PSUM tiles live) so
            # each transposed tile is consumed the moment it lands.
            def proj_T_group(dst, w_t, b_t, xt, tcis):
                pss = {}
                for tci in tcis:
                    ps = psB.tile([P, 2 * QCH], F32, tag="sc", name=f"ps{tci}")
                    pss[tci] = ps
                for dt in range(NDT):
                    for tci in tcis:
                        ts0 = tci * QCH
                        for dot in range(2):
                            col = slice(dot * QCH, (dot + 1) * QCH)
                            mm(
                                pss[tci][:, col],
                                w_t[:, dt, dot * P:(dot + 1) * P],
                                xt[dt][:, ts0:ts0 + QCH],
                                start=(dt == 0), stop=(dt == NDT - 1),
                            )
                for tci in tcis:
                    ps = pss[tci]
                    ts0 = tci * QCH
                    if dst is qt_t:
                        for dot in range(2):
                            nc.vector.tensor_scalar_add(
                                dst[:, dot, ts0:ts0 + QCH],
                                ps[:, dot * QCH:(dot + 1) * QCH],
                                b_t[:, dot, :])
                    else:  # kz_t: per-head 64-row slices, rest stays zero
                        for h in range(HPC):
                            rows = slice((h % 2) * 64, (h % 2) * 64 + 64)
                            dot = h // 2
                            nc.vector.tensor_scalar_add(
                                kz_t[rows, h, ts0:ts0 + QCH],
                                ps[rows, dot * QCH:(dot + 1) * QCH],
                                b_t[rows, dot, :])

            # ---- token-major V projection (one tile) ----
            # write head 2pr to pair-block cols 0:64, head 2pr+1 to 128:192
            def proj_v(tt):
                ps = psB.tile([P, DS], F32, tag="sc")
                for dt in range(NDT):
                    mm(
                        ps[:],
                        xt_v[dt][:, tt * P:(tt + 1) * P],
                        wv_t[:, dt, :],
                        start=(dt == 0), stop=(dt == NDT - 1),
                    )
                psv = ps.rearrange("p (pr h d) -> p pr h d", pr=2, h=2)
                bvv = bv_b.rearrange("p (pr h d) -> p pr h d", pr=2, h=2)
                nc.vector.tensor_add(
                    v2_t[:, tt, :, 0:DH], psv[:, :, 0, :], bvv[:, :, 0, :])
                nc.vector.tensor_add(
                    v2_t[:, tt, :, 2 * DH:VW], psv[:, :, 1, :], bvv[:, :, 1, :])

            # ---- attention phases ----
            # scores + exp for one kp step (2 k-blocks x 2 heads, plain)
            def ph1_kp(qc, pr, kp):
                qs = qc * QCH
                h0, h1 = 2 * pr, 2 * pr + 1
                sc0 = psB.tile([P, 2 * QCH], F32, tag="sc", name="sc0")
                sc1 = psB.tile([P, 2 * QCH], F32, tag="sc", name="sc1")
                for hsel, sc in ((h0, sc0), (h1, sc1)):
                    for j in range(2):
                        ks = (2 * kp + j) * P
                        col = slice(j * QCH, (j + 1) * QCH)
                        mm(
                            sc[:, col], kz_t[:, hsel, ks:ks + P],
                            qt_t[:, pr, qs:qs + QCH],
                            start=True, stop=True)
                e0 = expp.tile([P, 2 * QCH], F16, tag="exp", name="e0")
                e1 = expp.tile([P, 2 * QCH], F16, tag="exp", name="e1")
                nc.scalar.activation(e0[:], sc0[:], AF.Exp, scale=SCALE)
                nc.scalar.activation(e1[:], sc1[:], AF.Exp, scale=SCALE)
                return (e0, e1)

            # attn@V + free sums for a subset of k tiles, accumulating into
            # acc0/acc1 across the whole 16-kt sweep.  Plain M=128 matmuls:
            #   acc0 rows 0:64 = attn h0, rows 64:128 = sum(e0) per q
            #   acc1 rows 0:64 = sum(e1) per q, rows 64:128 = attn h1
            def ph2_chunk(qc, pr, etiles, kts, acc0, acc1):
                for kt in kts:
                    e0, e1 = etiles[kt // 2]
                    col = slice((kt % 2) * QCH, (kt % 2 + 1) * QCH)
                    st = (kt == 0)
                    sp = (kt == NKT - 1)
                    mm(acc0[:], v2_t[:, kt, pr, 0:P], e0[:, col],
                       start=st, stop=sp)
                    mm(acc1[:], v2_t[:, kt, pr, DH:DH + P], e1[:, col],
                       start=st, stop=sp)

            # DVE ops cannot cross partition bases on HW (and the custom
            # reciprocal only works at base 0), but the Scalar engine's
            # ACTIVATE-copy crosses bases fine and reads PSUM directly.
            # Crucially this keeps DMA out of the norm path - collective
            # (ReduceScatter) traffic hogs the DMA hardware for ~12us at a
            # time and would stall the pipeline behind it.
            def _norm_w(pr, qs, w, acc0, acc1):
                rc = rcpp.tile([P, QCH], F32, tag="rcp", name="rc")
                rc2 = rcpp.tile([P, QCH], F32, tag="rcp2", name="rc2")
                rc3 = rcpp.tile([P, QCH], F32, tag="rcp3", name="rc3")
                # h1 first: recip at base 0 straight from psum, then the
                # Scalar copy moves it up while h0's chain proceeds.
                nc.vector.reciprocal_approx_fast(rc3[0:64, 0:w],
                                                 acc1[0:64, :])
                nc.scalar.copy(rc3[64:P, 0:w], rc3[0:64, 0:w])
                # h0: ACT-copy sums down (psum rows 64:128 -> sbuf rows 0:64)
                nc.scalar.copy(rc2[0:64, 0:w], acc0[64:P, :])
                nc.vector.reciprocal_approx_fast(rc[0:64, 0:w],
                                                 rc2[0:64, 0:w])
                nc.vector.tensor_mul(
                    an_t[0:64, pr, qs:qs + w], acc0[0:64, :], rc[0:64, 0:w])
                nc.vector.tensor_mul(
                    an_t[64:P, pr, qs:qs + w], acc1[64:P, :], rc3[64:P, 0:w])

            def norm(qc, pr, acc0, acc1):
                _norm_w(pr, qc * QCH, QCH, acc0, acc1)

            def norm_sub(qc, pr, sub, a0, a1):
                """norm for one 128-q sub-chunk of the tail."""
                _norm_w(pr, qc * QCH + sub * P, P, a0, a1)

            def po_tt(qc, tt4):
                """one token tile of the output projection (plain matmuls)."""
                tt = qc * (QCH // P) + tt4
                po = psB.tile([P, 2 * QCH], F32, tag="sc")
                for half in range(2):
                    for pr in range(2):
                        mm(
                            po[:, half * QCH:(half + 1) * QCH],
                            an_t[:, pr, tt * P:(tt + 1) * P],
                            wo_t[:, pr, half * QCH:(half + 1) * QCH],
                            start=(pr == 0), stop=(pr == 1))
                ob = osbp.tile([P, D], F16, tag="osb")
                nc.vector.tensor_add(ob[:], po[:], bo_b[:])
                nc.sync.dma_start(
                    partial_cs[tt // 4][(tt % 4) * P:(tt % 4 + 1) * P, :],
                    ob[:])

            def rs_qc(qc):
                """per-chunk ReduceScatter (out DMAs issued at the end)."""
                if not with_cc:
                    return
                nc.gpsimd.collective_compute(
                    "ReduceScatter",
                    mybir.AluOpType.add,
                    replica_groups=REPLICA_GROUPS,
                    ins=[partial_cs[qc][:]],
                    outs=[rs_cs[qc][:]],
                )

            def out_dma(qc):
                if not with_cc:
                    return
                nc.sync.dma_start(out_d[qc * P:(qc + 1) * P, :],
                                  rs_cs[qc][:])

            def rs3_half(h):
                """qc=3 ReduceScatter in [256, D] halves: the first fires
                while the tail's remaining sub-chunks still compute."""
                if not with_cc:
                    return
                nc.gpsimd.collective_compute(
                    "ReduceScatter",
                    mybir.AluOpType.add,
                    replica_groups=REPLICA_GROUPS,
                    ins=[partial_cs[3][h * 2 * P:(h + 1) * 2 * P, :]],
                    outs=[rs3h_cs[h][:]],
                )
                nc.sync.dma_start(
                    out_d[3 * P + h * 64:3 * P + (h + 1) * 64, :],
                    rs3h_cs[h][:])



            # ---- emission schedule ----
            proj_T_group(kz_t, wk_t, bk_t, xt_k, [0, 1, 2])
            proj_T_group(kz_t, wk_t, bk_t, xt_k, [3])
            proj_T_group(qt_t, wq_t, bq_t, xt_q, [0, 1, 2])
            proj_T_group(qt_t, wq_t, bq_t, xt_q, [3])

            # first q-chunk's pr0 scores run while V transposes stream in
            et00 = [ph1_kp(0, 0, kp) for kp in range(NKT // 2)]
            for tt in range(NTT):
                proj_v(tt)



            # software pipeline: each window FRONT-LOADS ph2(prev) so that
            # norm(prev) completes mid-window - the next window's first
            # attn@V matmul (which reuses the acc banks) then never stalls.
            # ph1(cur) kp0-3 interleave to keep the Scalar exp stream fed;
            # kp4-7 + the output projection of qc-1 fill the window's tail.
            prev = (0, 0, et00)
            seq = [(qc, pr) for qc in range(NQC) for pr in range(2)][1:]
            for qc, pr in seq:
                acc0 = psC.tile([P, QCH], F32, tag="acc0", name="acc0")
                acc1 = psC.tile([P, QCH], F32, tag="acc1", name="acc1")
                # po(qc-1) runs at the START of the pr1 window (its an_t was
                # normalized a full window earlier - no wait) and its
                # ReduceScatter fires mid-window, keeping collectives off the
                # kernel tail.
                po_src = qc - 1 if (pr == 1 and qc >= 1) else None
                et = []
                et.append(ph1_kp(qc, pr, 0))
                if po_src is not None:
                    po_tt(po_src, 0)
                    po_tt(po_src, 1)
                et.append(ph1_kp(qc, pr, 1))
                if po_src is not None:
                    po_tt(po_src, 2)
                    po_tt(po_src, 3)
                ph2_chunk(prev[0], prev[1], prev[2], range(0, 8), acc0, acc1)
                et.append(ph1_kp(qc, pr, 2))
                if po_src is not None:
                    rs_qc(po_src)
                ph2_chunk(prev[0], prev[1], prev[2], range(8, 16), acc0, acc1)
                et.append(ph1_kp(qc, pr, 3))
                norm(prev[0], prev[1], acc0, acc1)
                for kp in range(4, 8):
                    et.append(ph1_kp(qc, pr, kp))
                prev = (qc, pr, et)

            # tail: the last chunk's attn@V in four 128-q sub-chunks, each
            # immediately normalized + output-projected + reduce-scattered
            # (per-token-tile RS quarters the final collective's latency).
            # Sub-accumulators borrow psB slots: [:, 0:128] of bank A and
            # [QCH:QCH+128] of bank B - start=True zeroing is bank-granular
            # and both banks belong solely to the slot.
            subacc = []
            for sub in range(4):
                slot = psB.tile([P, 2 * QCH], F32, tag="sc", name=f"fs{sub}")
                a0 = slot[:, 0:P]
                a1 = slot[:, QCH:QCH + P]
                for kt in range(NKT):
                    e0, e1 = prev[2][kt // 2]
                    col = slice((kt % 2) * QCH + sub * P,
                                (kt % 2) * QCH + (sub + 1) * P)
                    st = (kt == 0)
                    sp = (kt == NKT - 1)
                    mm(a0, v2_t[:, kt, 1, 0:P], e0[:, col],
                       start=st, stop=sp)
                    mm(a1, v2_t[:, kt, 1, DH:DH + P], e1[:, col],
                       start=st, stop=sp)
                subacc.append((a0, a1))
                # trail the previous sub's norm/po under this sub's matmuls
                if sub >= 1:
                    norm_sub(3, 1, sub - 1, *subacc[sub - 1])
                    po_tt(3, sub - 1)
                if sub == 2:
                    rs3_half(0)
            norm_sub(3, 1, 3, *subacc[3])
            po_tt(3, 3)
            rs3_half(1)
            out_dma(0)
            out_dma(1)
            out_dma(2)

    nc.compile()
    return nc


def _get_nc():
    global _CACHED_NC
    if _CACHED_NC is None:
        _CACHED_NC = _build_module()
    return _CACHED_NC


def _block_x(x):
    """[S, D] fp32 -> [NDT, S, P] fp16 contiguous dt-blocks."""
    return np.ascontiguousarray(
        x.reshape(S, NDT, P).transpose(1, 0, 2)).astype(np.float16)


def _make_in_maps(query, key, value, Wq, bq, Wk, bk, Wv, bv, Wo, bo):
    query = np.asarray(query, dtype=np.float32)
    key = np.asarray(key, dtype=np.float32)
    value = np.asarray(value, dtype=np.float32)
    Wq = np.asarray(Wq, dtype=np.float32)
    Wk = np.asarray(Wk, dtype=np.float32)
    Wv = np.asarray(Wv, dtype=np.float32)
    Wo = np.asarray(Wo, dtype=np.float32)
    bq = np.asarray(bq, dtype=np.float32)
    bk = np.asarray(bk, dtype=np.float32)
    bv = np.asarray(bv, dtype=np.float32)
    bo = np.asarray(bo, dtype=np.float32)

    xb = [(_block_x(query[b]), _block_x(key[b]), _block_x(value[b]))
          for b in range(B)]

    in_maps = []
    for c in range(NCORES):
        b = c // GPB
        g = c % GPB
        sl = slice(g * DS, (g + 1) * DS)
        in_maps.append({
            "xq": xb[b][0],
            "xk": xb[b][1],
            "xv": xb[b][2],
            "wq": Wq[:, sl].astype(np.float16),
            "wk": Wk[:, sl].astype(np.float16),
            "wv": Wv[:, sl].astype(np.float16),
            "wo": Wo[sl, :].astype(np.float16),
            "bq": bq[sl].reshape(DS, 1).copy(),
            "bk": bk[sl].reshape(DS, 1).copy(),
            "bv": bv[sl].reshape(1, DS).copy(),
            "bo": (bo if g == 0 else np.zeros_like(bo)).reshape(1, D).copy(),
        })
    return in_maps


def run(inputs, trace=False, trace_cores=None):
    """Run the SPMD kernel; returns (full_output, BassKernelResults)."""
    nc = _get_nc()
    in_maps = _make_in_maps(**inputs)
    res = run_bass_kernel_spmd(
        nc, in_maps, core_ids=list(range(NCORES)), trace=trace,
        trace_cores=trace_cores)
    out = np.empty((B, S, D), dtype=np.float32)
    for c in range(NCORES):
        b = c // GPB
        g = c % GPB
        o = res.results[c]["out"].astype(np.float32)
        for j in range(3):
            out[b, j * 512 + g * P:j * 512 + (g + 1) * P, :] = \
                o[j * P:(j + 1) * P, :]
        # qc=3 was reduce-scattered in two [256, D] halves (64 rows/core)
        for h in range(2):
            out[b, 3 * 512 + h * 256 + g * 64:
                   3 * 512 + h * 256 + (g + 1) * 64, :] = \
                o[3 * P + h * 64:3 * P + (h + 1) * 64, :]
    return out, res


def kernel(**inputs):
    out, _ = run(inputs, trace=False)
    return out


# revision 52
# speedup vs baseline: 1.0370x; 1.0143x over previous
"""Multi-head attention (B=2, S=2048, D=1024, H=16) on 8 Trainium2 NeuronCores.

Sharding: batch x head-group. Core c handles batch b = c//4 and heads
[4*(c%4), 4*(c%4)+4) (a 256-wide slice of the QKV projection output and the
matching 256-row slice of Wo). Each core computes its partial output
projection; a 4-way ReduceScatter per batch group sums the partials and
writes each core's [128, 1024] row block of the final output directly, which
the host reassembles.

Per-core dataflow (all matmul operands fp16, fp32 PSUM accumulation):
  - x fed pre-blocked from the host as [8, 2048, 128] contiguous dt-blocks;
    DMA transposes issued as [1024, 128] halves (~2x the throughput of a
    full [2048, 128] transpose on the xbar path).
  - Q^T, K^T feature-major [256, 2048]; K^T zero-padded per head (full-128
    contraction); V token-major with per-head-pair blocks [v_h0|ones|v_h1]
    (192 cols) so each attn@V matmul (M=128, plain mode) also accumulates the
    softmax denominators on the 64 partitions opposite the attn rows - the
    per-k sums cost zero extra PE cycles and arrive pre-broadcast.
  - Softmax without max-subtraction (exp via ScalarE, 1/sqrt(dh) folded in);
    projection bias/copies are Vector tensor_scalar/tensor_tensor ops.
    Normalization crosses the 64-partition boundary with Scalar ACTIVATE
    copies - the only engine that can shift partition bases on HW (DVE ops
    cannot, and DMA would queue behind ReduceScatter SDMA traffic for
    10-50us at a time).
  - No tile_position anywhere: a single plain PE mode, no drain semaphores.
  - Software pipeline: each window front-loads ph2 attn@V of the previous
    window (norm(prev) then completes mid-window, freeing the acc banks
    before the next window needs them) interleaved with ph1 scores+exp of
    the current window; po(qc-1) runs at the start of pr1 windows and its
    per-chunk ReduceScatter fires mid-window.  The last q-chunk's attn@V
    runs as four 128-q sub-chunks in borrowed psB slots, each sub's norm and
    output projection trailing under the next sub's matmuls, so the final
    ReduceScatter fires as early as possible.
"""

import numpy as np

import concourse.bass as bass  # noqa: F401  (engine namespaces via nc)
import concourse.mybir as mybir
import concourse.tile as tile
from concourse import bacc
from concourse.bass import _add_dep_helper
from concourse.bass_utils import run_bass_kernel_spmd

F32 = mybir.dt.float32
F16 = mybir.dt.float16
AF = mybir.ActivationFunctionType

B, S, D = 2, 2048, 1024
H, DH = 16, 64
NCORES = 8
GPB = 4                # cores per batch group
HPC = H // GPB         # heads per core
DS = HPC * DH          # 256: per-core slice of the projection output
P = 128
NDT = D // P           # 8 d_model tiles
NTT = S // P           # 16 token tiles
QCH = 512              # q-chunk (PSUM bank = 512 fp32)
NQC = S // QCH         # 4
NKT = S // P           # 16 k tiles
VW = 192               # per-head-pair V block: [v_h0 | ones | v_h1]
SCALE = float(1.0 / np.sqrt(DH))

REPLICA_GROUPS = [[0, 1, 2, 3], [4, 5, 6, 7]]

_CACHED_NC = None


def _build_module(with_cc=True):
    nc = bacc.Bacc("TRN2", target_bir_lowering=False, debug=False,
                   num_devices=NCORES)

    xq_d = nc.dram_tensor("xq", [NDT, S, P], F16, kind="ExternalInput")
    xk_d = nc.dram_tensor("xk", [NDT, S, P], F16, kind="ExternalInput")
    xv_d = nc.dram_tensor("xv", [NDT, S, P], F16, kind="ExternalInput")
    wq_d = nc.dram_tensor("wq", [D, DS], F16, kind="ExternalInput")
    wk_d = nc.dram_tensor("wk", [D, DS], F16, kind="ExternalInput")
    wv_d = nc.dram_tensor("wv", [D, DS], F16, kind="ExternalInput")
    wo_d = nc.dram_tensor("wo", [DS, D], F16, kind="ExternalInput")
    bq_d = nc.dram_tensor("bq", [DS, 1], F32, kind="ExternalInput")
    bk_d = nc.dram_tensor("bk", [DS, 1], F32, kind="ExternalInput")
    bv_d = nc.dram_tensor("bv", [1, DS], F32, kind="ExternalInput")
    bo_d = nc.dram_tensor("bo", [1, D], F32, kind="ExternalInput")

    out_d = nc.dram_tensor("out", [S // GPB, D], F16, kind="ExternalOutput")
    partial_cs = [nc.dram_tensor(f"partial{j}", [4 * P, D], F16)
                  for j in range(4)]
    rs_cs = [nc.dram_tensor(f"rs_out{j}", [P, D], F16)
             for j in range(3)]
    rs3h_cs = [nc.dram_tensor(f"rs3h{j}", [P // 2, D], F16)
               for j in range(2)]


    with tile.TileContext(nc) as tc:
        with (
            tc.tile_pool(name="cst", bufs=1) as cst,
            tc.tile_pool(name="xt", bufs=12) as xtp,
            tc.tile_pool(name="exp", bufs=26) as expp,
            tc.tile_pool(name="rcp", bufs=2) as rcpp,
            tc.tile_pool(name="osb", bufs=8) as osbp,
            tc.tile_pool(name="psB", bufs=3, space="PSUM") as psB,
            tc.tile_pool(name="psC", bufs=1, space="PSUM") as psC,
        ):
            # Total PE ordering: chain every matmul to its predecessor
            # (nosync = scheduling-order only). All matmuls are plain
            # 128x128 mode - no tile_position, no mode-switch drains.
            _real_matmul = nc.tensor.matmul
            _prev_mm = {"inst": None}

            def mm(out, lhsT, rhs, **kw):
                inst = _real_matmul(out, lhsT, rhs, **kw)
                if _prev_mm["inst"] is not None:
                    _add_dep_helper(
                        inst.ins, _prev_mm["inst"].ins,
                        sync=False, reason="pe-order")
                _prev_mm["inst"] = inst
                return inst

            # ---- constants (sync HWDGE queue, wk first) ----
            wq_t = cst.tile([P, NDT, DS], F16, tag="wq")
            wk_t = cst.tile([P, NDT, DS], F16, tag="wk")
            wv_t = cst.tile([P, NDT, DS], F16, tag="wv")
            wo_t = cst.tile([P, 2, D], F16, tag="wo")
            bq_t = cst.tile([P, 2, 1], F32, tag="bq")
            bk_t = cst.tile([P, 2, 1], F32, tag="bk")
            bv_row = cst.tile([1, DS], F32, tag="bvr")
            bo_row = cst.tile([1, D], F32, tag="bor")

            # wo/bo are not needed until the first output projection
            # (~115us in) - load them AFTER the transposes so the transpose
            # stream starts ~1.5us earlier.
            nc.sync.dma_start(wk_t[:], wk_d.rearrange("(a p) n -> p a n", p=P))
            nc.sync.dma_start(wq_t[:], wq_d.rearrange("(a p) n -> p a n", p=P))
            nc.sync.dma_start(bk_t[:], bk_d.rearrange("(a p) o -> p a o", p=P))
            nc.sync.dma_start(bq_t[:], bq_d.rearrange("(a p) o -> p a o", p=P))
            nc.sync.dma_start(bv_row[:], bv_d[:])
            nc.sync.dma_start(wv_t[:], wv_d.rearrange("(a p) n -> p a n", p=P))

            bv_b = cst.tile([P, DS], F32, tag="bvb")
            bo_b = cst.tile([P, D], F32, tag="bob")
            nc.gpsimd.partition_broadcast(bv_b[:], bv_row[:])

            # ---- activations: resident projections ----
            qt_t = cst.tile([P, 2, S], F16, tag="qt")   # Q^T  (pair, t)
            # K^T zero-padded per head: kz[:, h, :] has rows (h%2)*64..+64 =
            # K_h^T, other 64 rows zero -> full-K=128 scores matmuls.
            kz_t = cst.tile([P, HPC, S], F16, tag="kz")
            nc.vector.memset(kz_t[:], 0.0)
            # V token-major, per pair [v_h0 | ones | v_h1] (ones shared)
            v2_t = cst.tile([P, NTT, 2, VW], F16, tag="vt")
            nc.vector.memset(v2_t[:, :, :, DH:2 * DH], 1.0)
            an_t = cst.tile([P, 2, S], F16, tag="an")   # attn/sum ratio ^T

            # ---- transposed input tiles (DMA transpose, fp16) ----
            # [1024, 128] half-transposes run ~2x the throughput of full
            # [2048, 128] ones (1.30us vs ~2.9us per half-pair).
            def load_xt(x_d, half_major=False):
                tiles = [xtp.tile([P, S], F16, tag="xt", name=f"xt{dt}")
                         for dt in range(NDT)]
                hs = ([(h, dt) for h in range(2) for dt in range(NDT)]
                      if half_major else
                      [(h, dt) for dt in range(NDT) for h in range(2)])
                for h, dt in hs:
                    nc.sync.dma_start(
                        tiles[dt][:, h * 1024:(h + 1) * 1024],
                        x_d[dt, h * 1024:(h + 1) * 1024, :], transpose=True)
                return tiles

            xt_k = load_xt(xk_d)
            xt_q = load_xt(xq_d)
            xt_v = load_xt(xv_d, half_major=True)

            nc.sync.dma_start(wo_t[:], wo_d.rearrange("(a p) n -> p a n", p=P))
            nc.sync.dma_start(bo_row[:], bo_d[:])
            nc.gpsimd.partition_broadcast(bo_b[:], bo_row[:])

            # ---- feature-major projection: out^T[ds, t] (Q^T / K^T) ----
            # dt-interleaved across up to 3 q-chunks (3 PSUM tiles live) so
            # each transposed tile is consumed the moment it lands.
            def proj_T_group(dst, w_t, b_t, xt, tcis):
                pss = {}
                for tci in tcis:
                    ps = psB.tile([P, 2 * QCH], F32, tag="sc", name=f"ps{tci}")
                    pss[tci] = ps
                for dt in range(NDT):
                    for tci in tcis:
                        ts0 = tci * QCH
                        for dot in range(2):
                            col = slice(dot * QCH, (dot + 1) * QCH)
                            mm(
                                pss[tci][:, col],
                                w_t[:, dt, dot * P:(dot + 1) * P],
                                xt[dt][:, ts0:ts0 + QCH],
                                start=(dt == 0), stop=(dt == NDT - 1),
                            )
                for tci in tcis:
                    ps = pss[tci]
                    ts0 = tci * QCH
                    if dst is qt_t:
                        for dot in range(2):
                            nc.vector.tensor_scalar_add(
                                dst[:, dot, ts0:ts0 + QCH],
                                ps[:, dot * QCH:(dot + 1) * QCH],
                                b_t[:, dot, :])
                    else:  # kz_t: per-head 64-row slices, rest stays zero
                        for h in range(HPC):
                            rows = slice((h % 2) * 64, (h % 2) * 64 + 64)
                            dot = h // 2
                            nc.vector.tensor_scalar_add(
                                kz_t[rows, h, ts0:ts0 + QCH],
                                ps[rows, dot * QCH:(dot + 1) * QCH],
                                b_t[rows, dot, :])

            # ---- token-major V projection (one tile) ----
            # write head 2pr to pair-block cols 0:64, head 2pr+1 to 128:192
            def proj_v(tt):
                ps = psB.tile([P, DS], F32, tag="sc")
                for dt in range(NDT):
                    mm(
                        ps[:],
                        xt_v[dt][:, tt * P:(tt + 1) * P],
                        wv_t[:, dt, :],
                        start=(dt == 0), stop=(dt == NDT - 1),
                    )
                psv = ps.rearrange("p (pr h d) -> p pr h d", pr=2, h=2)
                bvv = bv_b.rearrange("p (pr h d) -> p pr h d", pr=2, h=2)
                nc.vector.tensor_add(
                    v2_t[:, tt, :, 0:DH], psv[:, :, 0, :], bvv[:, :, 0, :])
                nc.vector.tensor_add(
                    v2_t[:, tt, :, 2 * DH:VW], psv[:, :, 1, :], bvv[:, :, 1, :])

            # ---- attention phases ----
            # scores + exp for one kp step (2 k-blocks x 2 heads, plain)
            def ph1_kp(qc, pr, kp):
                qs = qc * QCH
                h0, h1 = 2 * pr, 2 * pr + 1
                sc0 = psB.tile([P, 2 * QCH], F32, tag="sc", name="sc0")
                sc1 = psB.tile([P, 2 * QCH], F32, tag="sc", name="sc1")
                for hsel, sc in ((h0, sc0), (h1, sc1)):
                    for j in range(2):
                        ks = (2 * kp + j) * P
                        col = slice(j * QCH, (j + 1) * QCH)
                        mm(
                            sc[:, col], kz_t[:, hsel, ks:ks + P],
                            qt_t[:, pr, qs:qs + QCH],
                            start=True, stop=True)
                e0 = expp.tile([P, 2 * QCH], F16, tag="exp", name="e0")
                e1 = expp.tile([P, 2 * QCH], F16, tag="exp", name="e1")
                nc.scalar.activation(e0[:], sc0[:], AF.Exp, scale=SCALE)
                nc.scalar.activation(e1[:], sc1[:], AF.Exp, scale=SCALE)
                return (e0, e1)

            # attn@V + free sums for a subset of k tiles, accumulating into
            # acc0/acc1 across the whole 16-kt sweep.  Plain M=128 matmuls:
            #   acc0 rows 0:64 = attn h0, rows 64:128 = sum(e0) per q
            #   acc1 rows 0:64 = sum(e1) per q, rows 64:128 = attn h1
            def ph2_chunk(qc, pr, etiles, kts, acc0, acc1):
                for kt in kts:
                    e0, e1 = etiles[kt // 2]
                    col = slice((kt % 2) * QCH, (kt % 2 + 1) * QCH)
                    st = (kt == 0)
                    sp = (kt == NKT - 1)
                    mm(acc0[:], v2_t[:, kt, pr, 0:P], e0[:, col],
                       start=st, stop=sp)
                    mm(acc1[:], v2_t[:, kt, pr, DH:DH + P], e1[:, col],
                       start=st, stop=sp)

            # DVE ops cannot cross partition bases on HW (and the custom
            # reciprocal only works at base 0), but the Scalar engine's
            # ACTIVATE-copy crosses bases fine and reads PSUM directly.
            # Crucially this keeps DMA out of the norm path - collective
            # (ReduceScatter) traffic hogs the DMA hardware for ~12us at a
            # time and would stall the pipeline behind it.
            def _norm_w(pr, qs, w, acc0, acc1):
                rc = rcpp.tile([P, QCH], F32, tag="rcp", name="rc")
                rc2 = rcpp.tile([P, QCH], F32, tag="rcp2", name="rc2")
                rc3 = rcpp.tile([P, QCH], F32, tag="rcp3", name="rc3")
                # h1 first: recip at base 0 straight from psum, then the
                # Scalar copy moves it up while h0's chain proceeds.
                nc.vector.reciprocal_approx_fast(rc3[0:64, 0:w],
                                                 acc1[0:64, :])
                nc.scalar.copy(rc3[64:P, 0:w], rc3[0:64, 0:w])
                # h0: ACT-copy sums down (psum rows 64:128 -> sbuf rows 0:64)
                nc.scalar.copy(rc2[0:64, 0:w], acc0[64:P, :])
                nc.vector.reciprocal_approx_fast(rc[0:64, 0:w],
                                                 rc2[0:64, 0:w])
                nc.vector.tensor_mul(
                    an_t[0:64, pr, qs:qs + w], acc0[0:64, :], rc[0:64, 0:w])
                nc.vector.tensor_mul(
                    an_t[64:P, pr, qs:qs + w], acc1[64:P, :], rc3[64:P, 0:w])

            def norm(qc, pr, acc0, acc1):
                _norm_w(pr, qc * QCH, QCH, acc0, acc1)

            def norm_sub(qc, pr, sub, a0, a1):
                """norm for one 128-q sub-chunk of the tail."""
                _norm_w(pr, qc * QCH + sub * P, P, a0, a1)

            def po_tt(qc, tt4):
                """one token tile of the output projection (plain matmuls)."""
                tt = qc * (QCH // P) + tt4
                # NOTE: a single N=1024 matmul (both D-halves) fails walrus
                # codegen - PSUM matmul writes may not span a bank (512 f32).
                po = psB.tile([P, 2 * QCH], F32, tag="sc")
                for half in range(2):
                    for pr in range(2):
                        mm(
                            po[:, half * QCH:(half + 1) * QCH],
                            an_t[:, pr, tt * P:(tt + 1) * P],
                            wo_t[:, pr, half * QCH:(half + 1) * QCH],
                            start=(pr == 0), stop=(pr == 1))
                ob = osbp.tile([P, D], F16, tag="osb")
                nc.vector.tensor_add(ob[:], po[:], bo_b[:])
                nc.sync.dma_start(
                    partial_cs[tt // 4][(tt % 4) * P:(tt % 4 + 1) * P, :],
                    ob[:])

            def rs_qc(qc):
                """per-chunk ReduceScatter (out DMAs issued at the end)."""
                if not with_cc:
                    return
                nc.gpsimd.collective_compute(
                    "ReduceScatter",
                    mybir.AluOpType.add,
                    replica_groups=REPLICA_GROUPS,
                    ins=[partial_cs[qc][:]],
                    outs=[rs_cs[qc][:]],
                )

            def out_dma(qc):
                if not with_cc:
                    return
                nc.sync.dma_start(out_d[qc * P:(qc + 1) * P, :],
                                  rs_cs[qc][:])

            def rs3_half(h):
                """qc=3 ReduceScatter in [256, D] halves: the first fires
                while the tail's remaining sub-chunks still compute.  The
                out DMA is NOT issued here - an RS-gated DMA would
                head-block the remaining partial writes on the sync queue."""
                if not with_cc:
                    return
                nc.gpsimd.collective_compute(
                    "ReduceScatter",
                    mybir.AluOpType.add,
                    replica_groups=REPLICA_GROUPS,
                    ins=[partial_cs[3][h * 2 * P:(h + 1) * 2 * P, :]],
                    outs=[rs3h_cs[h][:]],
                )

            def out3_dma(h):
                if not with_cc:
                    return
                nc.sync.dma_start(
                    out_d[3 * P + h * 64:3 * P + (h + 1) * 64, :],
                    rs3h_cs[h][:])



            # ---- emission schedule ----
            proj_T_group(kz_t, wk_t, bk_t, xt_k, [0, 1, 2])
            proj_T_group(kz_t, wk_t, bk_t, xt_k, [3])
            proj_T_group(qt_t, wq_t, bq_t, xt_q, [0, 1, 2])
            proj_T_group(qt_t, wq_t, bq_t, xt_q, [3])

            # first q-chunk's pr0 scores run while V transposes stream in
            et00 = [ph1_kp(0, 0, kp) for kp in range(NKT // 2)]
            for tt in range(NTT):
                proj_v(tt)



            # software pipeline: each window FRONT-LOADS ph2(prev) so that
            # norm(prev) completes mid-window - the next window's first
            # attn@V matmul (which reuses the acc banks) then never stalls.
            # ph1(cur) kp0-3 interleave to keep the Scalar exp stream fed;
            # kp4-7 + the output projection of qc-1 fill the window's tail.
            prev = (0, 0, et00)
            seq = [(qc, pr) for qc in range(NQC) for pr in range(2)][1:]
            for qc, pr in seq:
                acc0 = psC.tile([P, QCH], F32, tag="acc0", name="acc0")
                acc1 = psC.tile([P, QCH], F32, tag="acc1", name="acc1")
                # po(qc-1) runs at the START of the pr1 window (its an_t was
                # normalized a full window earlier - no wait) and its
                # ReduceScatter fires mid-window, keeping collectives off the
                # kernel tail.
                po_src = qc - 1 if (pr == 1 and qc >= 1) else None
                et = []
                et.append(ph1_kp(qc, pr, 0))
                if po_src is not None:
                    po_tt(po_src, 0)
                    po_tt(po_src, 1)
                et.append(ph1_kp(qc, pr, 1))
                if po_src is not None:
                    po_tt(po_src, 2)
                    po_tt(po_src, 3)
                ph2_chunk(prev[0], prev[1], prev[2], range(0, 8), acc0, acc1)
                et.append(ph1_kp(qc, pr, 2))
                if po_src is not None:
                    rs_qc(po_src)
                ph2_chunk(prev[0], prev[1], prev[2], range(8, 16), acc0, acc1)
                et.append(ph1_kp(qc, pr, 3))
                norm(prev[0], prev[1], acc0, acc1)
                for kp in range(4, 8):
                    et.append(ph1_kp(qc, pr, kp))
                prev = (qc, pr, et)

            # tail: the last chunk's attn@V in four 128-q sub-chunks, each
            # immediately normalized + output-projected + reduce-scattered
            # (per-token-tile RS quarters the final collective's latency).
            # Sub-accumulators borrow psB slots: [:, 0:128] of bank A and
            # [QCH:QCH+128] of bank B - start=True zeroing is bank-granular
            # and both banks belong solely to the slot.
            # out DMAs for chunks 0-1 drain now (their RS long done); chunk
            # 2's RS may still be in flight, so its DMA goes at the end.
            out_dma(0)
            out_dma(1)
            subacc = []
            for sub in range(4):
                slot = psB.tile([P, 2 * QCH], F32, tag="sc", name=f"fs{sub}")
                a0 = slot[:, 0:P]
                a1 = slot[:, QCH:QCH + P]
                for kt in range(NKT):
                    e0, e1 = prev[2][kt // 2]
                    col = slice((kt % 2) * QCH + sub * P,
                                (kt % 2) * QCH + (sub + 1) * P)
                    st = (kt == 0)
                    sp = (kt == NKT - 1)
                    mm(a0, v2_t[:, kt, 1, 0:P], e0[:, col],
                       start=st, stop=sp)
                    mm(a1, v2_t[:, kt, 1, DH:DH + P], e1[:, col],
                       start=st, stop=sp)
                subacc.append((a0, a1))
                # trail the previous sub's norm/po under this sub's matmuls
                if sub >= 1:
                    norm_sub(3, 1, sub - 1, *subacc[sub - 1])
                    po_tt(3, sub - 1)
                if sub == 2:
                    rs3_half(0)
            norm_sub(3, 1, 3, *subacc[3])
            po_tt(3, 3)
            rs3_half(1)
            out_dma(2)
            out3_dma(0)
            out3_dma(1)

    nc.compile()
    return nc


def _get_nc():
    global _CACHED_NC
    if _CACHED_NC is None:
        _CACHED_NC = _build_module()
    return _CACHED_NC


def _block_x(x):
    """[S, D] fp32 -> [NDT, S, P] fp16 contiguous dt-blocks."""
    return np.ascontiguousarray(
        x.reshape(S, NDT, P).transpose(1, 0, 2)).astype(np.float16)


def _make_in_maps(query, key, value, Wq, bq, Wk, bk, Wv, bv, Wo, bo):
    query = np.asarray(query, dtype=np.float32)
    key = np.asarray(key, dtype=np.float32)
    value = np.asarray(value, dtype=np.float32)
    Wq = np.asarray(Wq, dtype=np.float32)
    Wk = np.asarray(Wk, dtype=np.float32)
    Wv = np.asarray(Wv, dtype=np.float32)
    Wo = np.asarray(Wo, dtype=np.float32)
    bq = np.asarray(bq, dtype=np.float32)
    bk = np.asarray(bk, dtype=np.float32)
    bv = np.asarray(bv, dtype=np.float32)
    bo = np.asarray(bo, dtype=np.float32)

    xb = [(_block_x(query[b]), _block_x(key[b]), _block_x(value[b]))
          for b in range(B)]

    in_maps = []
    for c in range(NCORES):
        b = c // GPB
        g = c % GPB
        sl = slice(g * DS, (g + 1) * DS)
        in_maps.append({
            "xq": xb[b][0],
            "xk": xb[b][1],
            "xv": xb[b][2],
            "wq": Wq[:, sl].astype(np.float16),
            "wk": Wk[:, sl].astype(np.float16),
            "wv": Wv[:, sl].astype(np.float16),
            "wo": Wo[sl, :].astype(np.float16),
            "bq": bq[sl].reshape(DS, 1).copy(),
            "bk": bk[sl].reshape(DS, 1).copy(),
            "bv": bv[sl].reshape(1, DS).copy(),
            "bo": (bo if g == 0 else np.zeros_like(bo)).reshape(1, D).copy(),
        })
    return in_maps


def run(inputs, trace=False, trace_cores=None):
    """Run the SPMD kernel; returns (full_output, BassKernelResults)."""
    nc = _get_nc()
    in_maps = _make_in_maps(**inputs)
    res = run_bass_kernel_spmd(
        nc, in_maps, core_ids=list(range(NCORES)), trace=trace,
        trace_cores=trace_cores)
    out = np.empty((B, S, D), dtype=np.float32)
    for c in range(NCORES):
        b = c // GPB
        g = c % GPB
        o = res.results[c]["out"].astype(np.float32)
        for j in range(3):
            out[b, j * 512 + g * P:j * 512 + (g + 1) * P, :] = \
                o[j * P:(j + 1) * P, :]
        # qc=3 was reduce-scattered in two [256, D] halves (64 rows/core)
        for h in range(2):
            out[b, 3 * 512 + h * 256 + g * 64:
                   3 * 512 + h * 256 + (g + 1) * 64, :] = \
                o[3 * P + h * 64:3 * P + (h + 1) * 64, :]
    return out, res


def kernel(**inputs):
    out, _ = run(inputs, trace=False)
    return out
